# revision 6
# baseline (speedup 1.0000x reference)
"""Multi-head attention layer (B=8, N=1024, E=1024, H=16, D=64) on 8 TRN2
NeuronCores, data-parallel over batch (one batch element per core, weights
replicated, no collectives).

Per-core algorithm (all matmuls bf16 inputs / fp32 PSUM accumulation):
  1. qkT[o, n]  = (w_qk @ x_b^T)         feature-major Q^T,K^T  (scale folded
                                          into the Q weights/bias on host)
  2. v[n, o]    = x_b @ w_v^T + v_b       token-major V, stored with a 65-col
                                          per-head stride whose 65th column
                                          is 1.0 (softmax-sum trick)
  3. per (head, i-half): scoresT[j, i] = k_j . q_i  -> exp on ScalarE ->
     probsT bf16; attnT[d|sum, i] = [V_h | 1]^T @ probsT (the extra lhsT
     column makes row 64 the softmax denominator). Normalize with
     reciprocal_approx_fast + DMA partition-broadcast + tensor_mul.
  4. y[i, o] = attnT_norm^T @ w_o^T + o_b  (biases applied via K=1 ones-row
     matmuls accumulated into PSUM).

scoresT orientation means no transposes anywhere in the chain.
exp is applied without max-subtraction: scores ~ N(0, 1/9), |s| < ~4.
"""

import os

import ml_dtypes
import numpy as np

B, N, E, H, D = 8, 1024, 1024, 16, 64
P = 128
KE = E // P  # contraction tiles over embedding dim
NT = N // P  # token tiles
MQK = 2 * E // P  # qkT output row-tiles
DP1 = D + 1  # per-head V columns incl. the ones column

TRACE = os.environ.get("BASS_KERNEL_TRACE", "0") == "1"
LAST_EXEC_NS = None
LAST_RESULT = None

_COMPILED = None


def _build():
    import concourse.bass as bass
    import concourse.tile as tile
    from concourse import bacc, mybir

    f32 = mybir.dt.float32
    bf16 = mybir.dt.bfloat16
    AF = mybir.ActivationFunctionType
    MS = bass.MemorySpace

    nc = bacc.Bacc(
        "TRN2", target_bir_lowering=False, debug=False, enable_asserts=True
    )

    xT_d = nc.dram_tensor("xT", [E, N], bf16, kind="ExternalInput")
    wqkT_d = nc.dram_tensor("wqkT", [E, 2 * E], bf16, kind="ExternalInput")
    qkb_d = nc.dram_tensor("qkb", [P, MQK], f32, kind="ExternalInput")
    wvT_d = nc.dram_tensor("wvT", [E, E], bf16, kind="ExternalInput")
    vb_d = nc.dram_tensor("vb", [1, E], bf16, kind="ExternalInput")
    woT_d = nc.dram_tensor("woT", [E, E], bf16, kind="ExternalInput")
    ob_d = nc.dram_tensor("ob", [1, E], bf16, kind="ExternalInput")
    y_d = nc.dram_tensor("y", [N, E], f32, kind="ExternalOutput")

    with tile.TileContext(nc) as tc, tc.tile_pool(name="persist", bufs=1) as persist:
        # --- persistent tiles (live for the whole kernel) ---
        qkT = [
            persist.tile([P, N], bf16, tag=f"qkT{m}", name=f"qkT{m}")
            for m in range(MQK)
        ]
        v_sb = [
            persist.tile([P, H * DP1], bf16, tag=f"v{m}", name=f"v{m}")
            for m in range(NT)
        ]
        attnT = [
            [
                persist.tile([P, 512], bf16, tag=f"attnT{k}_{ih}", name=f"attnT{k}_{ih}")
                for ih in range(2)
            ]
            for k in range(KE)
        ]
        woT = [
            persist.tile([P, E], bf16, tag=f"woT{k}", name=f"woT{k}")
            for k in range(KE)
        ]
        ones_row = persist.tile([1, P], bf16, tag="ones", name="ones")
        qkb_sb = persist.tile([P, MQK], f32, tag="qkb", name="qkb_sb")
        vb_sb = persist.tile([1, E], bf16, tag="vb", name="vb_sb")
        ob_sb = persist.tile([1, E], bf16, tag="ob", name="ob_sb")

        nc.vector.memset(ones_row[:], 1.0)
        nc.sync.dma_start(qkb_sb[:], qkb_d[:, :])
        nc.sync.dma_start(vb_sb[:], vb_d[:, :])
        nc.sync.dma_start(ob_sb[:], ob_d[:, :])
        for k in range(KE):
            nc.sync.dma_start(woT[k][:], woT_d[k * P : (k + 1) * P, :])

        # ---------------- stages 1+2 ----------------
        with (
            tc.tile_pool(name="ld", bufs=1) as ld,
            tc.tile_pool(name="ps12", bufs=3, space=MS.PSUM) as ps12,
        ):
            xT = [
                ld.tile([P, N], bf16, tag=f"xT{k}", name=f"xT{k}") for k in range(KE)
            ]
            wqkT = [
                ld.tile([P, 2 * E], bf16, tag=f"wqkT{k}", name=f"wqkT{k}")
                for k in range(KE)
            ]
            wvT = [
                ld.tile([P, E], bf16, tag=f"wvT{k}", name=f"wvT{k}")
                for k in range(KE)
            ]
            for k in range(KE):
                nc.sync.dma_start(xT[k][:], xT_d[k * P : (k + 1) * P, :])
                nc.sync.dma_start(wqkT[k][:], wqkT_d[k * P : (k + 1) * P, :])
                nc.sync.dma_start(wvT[k][:], wvT_d[k * P : (k + 1) * P, :])

            # stage 1: qkT[m] = w_qkT[:, m].T @ xT  (+ per-partition bias)
            for m in range(MQK):
                ps = ps12.tile([P, N], f32, tag="ps12", name="ps12")
                for nh in range(2):
                    nsl = slice(nh * 512, (nh + 1) * 512)
                    for k in range(KE):
                        nc.tensor.matmul(
                            ps[:, nsl],
                            wqkT[k][:, m * P : (m + 1) * P],
                            xT[k][:, nsl],
                            start=(k == 0),
                            stop=(k == KE - 1),
                        )
                nc.scalar.activation(
                    qkT[m][:], ps[:], AF.Identity, bias=qkb_sb[:, m : m + 1]
                )

            # stage 2: v[m] = xT[:, m].T @ wvT (+ v_b via ones-row matmul)
            for m in range(NT):
                ps = ps12.tile([P, N], f32, tag="ps12", name="ps12")
                for nh in range(2):
                    nsl = slice(nh * 512, (nh + 1) * 512)
                    for k in range(KE):
                        nc.tensor.matmul(
                            ps[:, nsl],
                            xT[k][:, m * P : (m + 1) * P],
                            wvT[k][:, nsl],
                            start=(k == 0),
                            stop=False,
                        )
                    nc.tensor.matmul(
                        ps[:, nsl],
                        ones_row[0:1, 0:P],
                        vb_sb[0:1, nsl],
                        start=False,
                        stop=True,
                    )
                # scatter into the 65-col-per-head layout; 65th col stays 1.0
                src3 = ps[:].rearrange("p (h c) -> p h c", c=D)
                dst3 = v_sb[m][:].rearrange("p (h c) -> p h c", c=DP1)
                nc.scalar.copy(dst3[:, :, 0:D], src3)
                nc.vector.memset(dst3[:, :, D : D + 1], 1.0)

        # ---------------- stages 3+4 ----------------
        with (
            tc.tile_pool(name="probs", bufs=18) as probs_pool,
            tc.tile_pool(name="psc", bufs=2, space=MS.PSUM) as psc_pool,
            tc.tile_pool(name="pat", bufs=2, space=MS.PSUM) as pat_pool,
            tc.tile_pool(name="staged", bufs=4) as staged_pool,
            tc.tile_pool(name="sums", bufs=4) as sums_pool,
            tc.tile_pool(name="ps4", bufs=1, space=MS.PSUM) as ps4_pool,
            tc.tile_pool(name="ysb", bufs=2) as y_pool,
            tc.tile_pool(name="dram", bufs=4, space=MS.DRAM) as dram_pool,
        ):

            def scores_block(hp, ih):
                """scoresT + exp for head pair (2hp, 2hp+1), query half ih.
                Returns the 8 probsT tiles [128 j, 512(A)|512(B)]."""
                isl = slice(ih * 512, (ih + 1) * 512)
                qt, kt = qkT[hp], qkT[8 + hp]
                pts = []
                for jt in range(NT):
                    ps = psc_pool.tile([P, N], f32, tag="psc", name="psc")
                    jsl = slice(jt * P, (jt + 1) * P)
                    # two heads row-packed: array rows 0-63 / 64-127
                    nc.tensor.matmul(
                        ps[:, 0:512], kt[0:64, jsl], qt[0:64, isl],
                        start=True, stop=True,
                    )
                    nc.tensor.matmul(
                        ps[:, 512:1024], kt[64:128, jsl], qt[64:128, isl],
                        start=True, stop=True,
                    )
                    pt = probs_pool.tile([P, N], bf16, tag="probs", name="probs")
                    nc.scalar.activation(pt[:], ps[:], AF.Exp)
                    pts.append(pt)
                return pts

            def av_block(hp, ih, pts):
                """AV + softmax-normalize for head pair; writes attnT."""
                sums = sums_pool.tile([2, 512], f32, tag="sums", name="sums")
                recip = sums_pool.tile([2, 512], f32, tag="recip", name="recip")
                stg = []
                for hs in range(2):
                    h = 2 * hp + hs
                    pa = pat_pool.tile([DP1, 512], f32, tag="pat", name="pat")
                    for jt in range(NT):
                        nc.tensor.matmul(
                            pa[:],
                            v_sb[jt][:, h * DP1 : (h + 1) * DP1],
                            pts[jt][:, hs * 512 : (hs + 1) * 512],
                            start=(jt == 0),
                            stop=(jt == NT - 1),
                        )
                    st = staged_pool.tile([DP1, 512], f32, tag="staged", name="staged")
                    nc.vector.tensor_copy(st[:], pa[:])
                    nc.sync.dma_start(sums[hs : hs + 1, :], st[D : D + 1, :])
                    stg.append(st)
                nc.vector.reciprocal_approx_fast(recip[:], sums[:])
                # partition-broadcast recip rows via a DRAM bounce (SBUF
                # sources cannot have partition-stride 0, DRAM sources can)
                rd = dram_pool.tile([2, 512], f32, tag="recip_dram", name="recip_dram")
                nc.sync.dma_start(rd[:], recip[:])
                for hs in range(2):
                    bc = staged_pool.tile([D, 512], f32, tag="bcast", name="bcast")
                    nc.gpsimd.dma_start(
                        bc[:], rd[hs : hs + 1, :].to_broadcast((D, 512))
                    )
                    base = hs * 64
                    nc.vector.tensor_mul(
                        attnT[hp][ih][base : base + 64, :], stg[hs][0:D, :], bc[:]
                    )

            def out_proj(ih):
                for c in range(4):
                    mi = ih * 4 + c
                    ps = ps4_pool.tile([P, E], f32, tag="ps4", name="ps4")
                    for nh in range(2):
                        nsl = slice(nh * 512, (nh + 1) * 512)
                        for k in range(KE):
                            nc.tensor.matmul(
                                ps[:, nsl],
                                attnT[k][ih][:, c * P : (c + 1) * P],
                                woT[k][:, nsl],
                                start=(k == 0),
                                stop=False,
                            )
                        nc.tensor.matmul(
                            ps[:, nsl],
                            ones_row[0:1, 0:P],
                            ob_sb[0:1, nsl],
                            start=False,
                            stop=True,
                        )
                    ysb = y_pool.tile([P, E], f32, tag="ysb", name="ysb")
                    nc.scalar.copy(ysb[:], ps[:])
                    nc.sync.dma_start(y_d[mi * P : (mi + 1) * P, :], ysb[:])

            # one-block software pipeline: scores(b+1) is emitted before
            # AV(b) so the PE has work while ScalarE exps block b.
            prev = None
            for ih in range(2):
                for hp in range(8):
                    pts = scores_block(hp, ih)
                    if prev is not None:
                        av_block(*prev)
                    prev = (hp, ih, pts)
            av_block(*prev)
            out_proj(0)
            out_proj(1)

    nc.compile()
    return nc


def _prep_inputs(x, qkv_w, qkv_b, out_w, out_b):
    """Host-side shard + layout prep. One batch element per core."""
    bf = ml_dtypes.bfloat16
    scale = np.float32(D ** -0.5)

    wq = (qkv_w[:E] * scale).astype(np.float32)
    wk = qkv_w[E : 2 * E]
    wv = qkv_w[2 * E :]
    wqkT = np.concatenate([wq, wk], axis=0).T.astype(bf)  # [E, 2E]
    wvT = np.ascontiguousarray(wv.T).astype(bf)  # [E, E]
    woT = np.ascontiguousarray(out_w.T).astype(bf)  # [E, E]

    qkb = np.concatenate([qkv_b[:E] * scale, qkv_b[E : 2 * E]]).astype(np.float32)
    qkb = np.ascontiguousarray(qkb.reshape(MQK, P).T)  # [P, MQK]
    vb = qkv_b[2 * E :].reshape(1, E).astype(bf)
    ob = out_b.reshape(1, E).astype(bf)

    in_maps = []
    for b in range(B):
        xT = np.ascontiguousarray(x[b].T).astype(bf)  # [E, N]
        in_maps.append(
            {
                "xT": xT,
                "wqkT": wqkT,
                "qkb": qkb,
                "wvT": wvT,
                "vb": vb,
                "woT": woT,
                "ob": ob,
            }
        )
    return in_maps


def _ensure_ntff_hook():
    """bass_utils trace=True under axon needs antenv.axon_hooks, which this
    image's antenv lacks. Inject an equivalent module backed by the ctypes
    NTFF profile hook from trn_agent_boot."""
    import sys
    import types

    try:
        from antenv.axon_hooks import get_axon_ntff_profile_hook  # noqa: F401

        return
    except ImportError:
        pass
    try:
        from trn_agent_boot.trn_boot import _ntff_profile_via_ctypes

        hook = _ntff_profile_via_ctypes("/opt/axon/libaxon_pjrt.so")
    except Exception:
        hook = None
    mod = types.ModuleType("antenv.axon_hooks")
    mod.get_axon_ntff_profile_hook = lambda: hook
    sys.modules["antenv.axon_hooks"] = mod


def kernel(x, qkv_w, qkv_b, out_w, out_b):
    global _COMPILED, LAST_EXEC_NS, LAST_RESULT
    from concourse.bass_utils import run_bass_kernel_spmd

    if TRACE:
        _ensure_ntff_hook()

    if _COMPILED is None:
        _COMPILED = _build()
    nc = _COMPILED

    in_maps = _prep_inputs(
        np.asarray(x, np.float32),
        np.asarray(qkv_w, np.float32),
        np.asarray(qkv_b, np.float32),
        np.asarray(out_w, np.float32),
        np.asarray(out_b, np.float32),
    )

    res = run_bass_kernel_spmd(nc, in_maps, core_ids=list(range(B)), trace=TRACE)
    LAST_RESULT = res
    LAST_EXEC_NS = res.exec_time_ns

    y = np.stack([np.asarray(res.results[c]["y"]) for c in range(B)], axis=0)
    return y.astype(np.float32)
